# revision 40
# baseline (speedup 1.0000x reference)
"""GroupGMM Trainium2 kernel (fp8 DoubleRow version).

Computes, for B=8192 samples with soft group-mixture weights over G=32 groups:
    logits = einsum("bi,gio,bg->bo", x, W_pi, g) + g @ b_pi        [B, 16]
    loc    = einsum(... W_mu ...)   + g @ b_mu                     [B, 512]
    scale  = softplus(einsum(... W_sigma ...) + g @ b_sigma)+1e-7  [B, 512]
    out    = concat([logits, loc, scale], -1)                      [B, 1040]

Strategy: data-parallel over batch across 8 NeuronCores (1024 rows each).
The group einsum folds into one matmul with contraction K = G*I = 16384 via
z[(g,i),b] = g[b,g]*x[b,i]. z and the weights are fp8e4 (scaled by 8 resp.
16 to dodge fp8 subnormals; the 1/128 is folded into the drains), so the PE
runs DoubleRow fp8 matmuls: 256-deep contraction per instruction at 0.5
cycles/row — 4x the bf16 matmul rate.

Three column passes over the 64 double-K tiles, with z resident in SBUF
after pass 1: sigma (8 PSUM banks, one per 128-sample chunk), then mu
(banks recycle as sigma drains), then pi (8 chunks packed in one recycled
bank). Pass 1 would be DMA-bound (z 16.8MB + W_sigma 8.4MB vs ~56us of
matmul), so z for 13 of the 32 groups is built on-chip instead of
streamed: the otherwise-idle Pool engine replicates gate rows across
partitions (partition_broadcast) and DVE (10 groups) plus Pool itself
(3 groups) multiply them into x^T tiles. PSUM accumulation commutes, so
the dk-tiles are processed in a host-chosen PERMUTED order (the host lays
W and z out in processing order): a greedy schedule interleaves streamed
and generated tiles so the DMA stream, DVE, and Pool all stay just ahead
of the PE. The g@b bias terms are accumulated directly in PSUM by a small
bf16 matmul (K=32) that opens each bank's accumulation group. The last
dk-positions of each pass run chunk-major so per-chunk stops (drains,
bank takeovers) stagger instead of bursting. softplus = Ln(Exp(v/128)+1)
on ACT; mu drains split ACT/DVE into bf16; outputs leave in four batched
bf16 stores and the host casts to f32.
"""

import numpy as np
import ml_dtypes

import concourse.bass as bass
import concourse.tile as tile
from concourse import bacc, mybir
from concourse.bass_utils import run_bass_kernel_spmd

B, I, G, C, D = 8192, 512, 32, 16, 32
CD = C * D                      # 512
OUT_W = C + 2 * CD              # 1040
NCORES = 8
BLOC = B // NCORES              # 1024
KTOT = G * I                    # 16384
NDK = KTOT // 256               # 64 double-K tiles (256-deep each)
NMC = BLOC // 128               # 8 sample chunks per core
ZS, WS = 8.0, 16.0              # fp8 pre-scales; drains divide by ZS*WS
SC = ZS * WS

# On-chip-generated groups (renumbering is free: the host permutes layouts).
N_DVE_G = 10
N_POOL_G = 3
STREAM_G = list(range(G - N_DVE_G - N_POOL_G))        # groups 0..18
DVE_G = list(range(len(STREAM_G), len(STREAM_G) + N_DVE_G))    # 19..28
POOL_G = list(range(DVE_G[-1] + 1, G))                         # 29..31
N_S = 2 * len(STREAM_G)         # 38 streamed dk-tiles

BF16 = mybir.dt.bfloat16
F32 = mybir.dt.float32
FP8 = mybir.dt.float8e4
DR = mybir.MatmulPerfMode.DoubleRow

_cache: dict = {}


def _schedule():
    """Greedy processing order for pass 1. Returns (perm, roles) where
    perm[pos] = dk-tile index (dt) processed at position pos and
    roles[pos] in {'S','D','P'}. Streamed dts are 0..N_S-1 in order; DVE
    dts are 2*DVE_G[0].. in order; Pool dts likewise. Cost constants are
    scheduling heuristics only — correctness never depends on them."""
    PE0, PEDT = 4.6, 0.875      # PE start and per-position matmul time
    WDMA, ZDMA = 0.364, 0.728   # us per position of W / of streamed z
    DVE0, DVEG = 7.8, 2.25       # DVE first-gen start, per-gen time
    # Pool per-gen cost is padded for the ~1.4us broadcasts it interleaves.
    POOL0, POOLG = 7.6, 6.3
    ns, nd, np_ = 0, 0, 0
    dma_t, pe_t = 1.0, PE0
    dve_t, pool_t = DVE0, POOL0
    perm, roles = [], []
    for pos in range(NDK):
        # Generated tiles are rate-limited: consume them the moment they
        # are ready (it costs nothing and spares stream DMA for later);
        # fall back to the stream, then to whatever finishes first.
        cand = []
        if nd < 2 * N_DVE_G:
            cand.append((max(dve_t + DVEG, dma_t + WDMA, pe_t), 0, "D"))
        if np_ < 2 * N_POOL_G:
            cand.append((max(pool_t + POOLG, dma_t + WDMA, pe_t), 1, "P"))
        if ns < N_S:
            cand.append((max(dma_t + ZDMA + WDMA, pe_t), 2, "S"))
        free = [c for c in cand if c[0] <= pe_t + PEDT + 1e-9]
        ready, _, role = min(free) if free else min(cand)
        roles.append(role)
        if role == "S":
            perm.append(ns)
            ns += 1
            dma_t += ZDMA + WDMA
        elif role == "D":
            perm.append(2 * DVE_G[0] + nd)
            nd += 1
            dve_t = max(dve_t, pe_t - DVEG) + DVEG
            dma_t += WDMA
        else:
            perm.append(2 * POOL_G[0] + np_)
            np_ += 1
            pool_t = max(pool_t, pe_t - POOLG) + POOLG
            dma_t += WDMA
        pe_t = max(pe_t + PEDT, ready)
    return perm, roles


PERM, ROLES = _schedule()
POS_OF_DT = {dt: pos for pos, dt in enumerate(PERM)}


def _z_runs():
    """Contiguous streamed-position runs, capped at 4, first two capped at
    2 for a fast pipeline start. Streamed positions map to consecutive
    z_d indices, so every run is one contiguous DMA on both sides."""
    runs = []
    pos = 0
    while pos < NDK:
        if ROLES[pos] != "S":
            pos += 1
            continue
        end = pos
        cap = 2 if len(runs) < 4 else 4
        while (end + 1 < NDK and ROLES[end + 1] == "S" and end + 1 - pos < cap):
            end += 1
        runs.append((pos, end + 1))
        pos = end + 1
    return runs


def _build_program():
    if "nc" in _cache:
        return _cache["nc"]
    from contextlib import ExitStack

    nc = bacc.Bacc("TRN2", target_bir_lowering=False, debug=False)

    # Host tensors are "partition-major" [128, ...] and already permuted
    # into processing order; z_d holds only the streamed positions.
    z_d = nc.dram_tensor("z", [128, N_S, 2, BLOC], FP8, kind="ExternalInput")
    wmu_d = nc.dram_tensor("wmu", [128, NDK, 2, CD], FP8, kind="ExternalInput")
    wsg_d = nc.dram_tensor("wsg", [128, NDK, 2, CD], FP8, kind="ExternalInput")
    wpi_d = nc.dram_tensor("wpi", [128, NDK, 2, C], FP8, kind="ExternalInput")
    xt_d = nc.dram_tensor("xt", [128, 4, BLOC], FP8, kind="ExternalInput")
    # gate rows (bf16, for staging/broadcast) packed with the fp8
    # DoubleRow-layout gate/bias operands for the PSUM bias matmuls.
    gtb_d = nc.dram_tensor("gtb", [G, BLOC], BF16, kind="ExternalInput")
    gb8_d = nc.dram_tensor("gb8", [16, 2, BLOC + OUT_W], FP8,
                           kind="ExternalInput")
    out_d = nc.dram_tensor("out", [BLOC, OUT_W], BF16, kind="ExternalOutput")

    # Position of each engine's i-th generated tile, and first-gen position
    # per on-chip group (for broadcast pacing).
    dve_pos = [p for p in range(NDK) if ROLES[p] == "D"]
    pool_pos = [p for p in range(NDK) if ROLES[p] == "P"]
    gen_pos = {}
    for i, p in enumerate(dve_pos):
        gen_pos[2 * DVE_G[0] + i] = p
    for i, p in enumerate(pool_pos):
        gen_pos[2 * POOL_G[0] + i] = p

    with tile.TileContext(nc) as tc, ExitStack() as ctx:
        res = ctx.enter_context(tc.tile_pool(name="res", bufs=1))
        gp = ctx.enter_context(tc.tile_pool(name="gp", bufs=6))
        wp = ctx.enter_context(tc.tile_pool(name="wp", bufs=3))
        op = ctx.enter_context(tc.tile_pool(name="op", bufs=1))
        pp = ctx.enter_context(tc.tile_pool(name="pp", bufs=1, space="PSUM"))

        gb8 = res.tile([16, 2, BLOC + OUT_W], FP8, name="gb8", tag="gb8")
        nc.sync.dma_start(gb8[:], gb8_d[:])
        gt8 = gb8[:, :, 0:BLOC]
        bpi = gb8[:, :, BLOC:BLOC + C]
        bmu = gb8[:, :, BLOC + C:BLOC + C + CD]
        bsg = gb8[:, :, BLOC + C + CD:]

        zt = res.tile([128, NDK, 2, BLOC], FP8, name="zt", tag="zt")
        xtr = res.tile([128, 4, BLOC], FP8, name="xtr", tag="xtr")

        # Sigma-pass stream on the in-order sync queue, keyed by the
        # position that first needs each item.
        QD = 4
        NQ = NDK // QD
        onchip = DVE_G + POOL_G
        first_need = {gg: min(gen_pos[2 * gg], gen_pos[2 * gg + 1])
                      for gg in onchip}
        stream = []
        sidx = {}
        for i, (a, bnd) in enumerate(_z_runs()):
            stream.append((a, 0, ("z", a, bnd)))
        for q in range(NQ):
            stream.append((q * QD + 0.5, 0, ("w", q)))
        # x^T feeds every on-chip multiply: get it in right behind the
        # first z/w items, before the stream saturates.
        stream.append((1.5, 0, ("xt", 0)))
        stream.append((3.0, 0, ("xt", 1)))
        # Gate-row staging (2KB each): ahead of the group's broadcast,
        # paced so a 4-deep ring never blocks the queue. The two earliest
        # groups are staged via SWDGE on the idle Pool queue instead (no
        # HWDGE slot, no delay to the z/w stream).
        early_gs = sorted(onchip, key=lambda gg: first_need[gg])[:2]
        for gg in onchip:
            if gg not in early_gs:
                stream.append((max(1.2, first_need[gg] - 7), 1, ("gs", gg)))
        stream.sort(key=lambda s: (s[0], s[1]))

        wsg_t = {}
        gsr = {}
        for _, _, item in stream:
            if item[0] == "z":
                _, a, bnd = item
                s0 = sum(1 for p in range(a) if ROLES[p] == "S")
                nc.sync.dma_start(zt[:, a:bnd], z_d[:, s0:s0 + (bnd - a)])
            elif item[0] == "w":
                q = item[1]
                wt = wp.tile([128, QD, 2, CD], FP8, name=f"wsg{q}", tag="w")
                nc.sync.dma_start(wt[:], wsg_d[:, q * QD:(q + 1) * QD])
                wsg_t[q] = wt
            elif item[0] == "gs":
                gg = item[1]
                scr = gp.tile([1, BLOC], BF16, name=f"gsr{gg}", tag="gsr",
                              bufs=5)
                nc.sync.dma_start(scr[:], gtb_d[gg:gg + 1, :])
                gsr[gg] = scr
            else:
                h = item[1]
                nc.sync.dma_start(xtr[:, 2 * h:2 * h + 2],
                                  xt_d[:, 2 * h:2 * h + 2])
        # Loads first needed later, queued behind the sigma stream.
        wpi = res.tile([128, NDK, 2, C], FP8, name="wpi", tag="wpi")
        nc.sync.dma_start(wpi[:], wpi_d[:])

        # On-chip z generation, emitted on Pool in need order: DVE-group
        # broadcasts must lead DVE's serial gen queue; Pool's own pairs
        # (broadcast + two multiplies) slot in between by position.
        def gen_item(eng, gg, t, half):
            dt = 2 * gg + half
            ib = 2 * half
            gb = t[:].unsqueeze(1).broadcast_to([128, 2, BLOC])
            eng.tensor_mul(zt[:, gen_pos[dt]], xtr[:, ib:ib + 2], gb)

        for gg in early_gs:
            scr = gp.tile([1, BLOC], BF16, name=f"gsre{gg}", tag="gsre",
                          bufs=2)
            nc.gpsimd.dma_start(scr[:], gtb_d[gg:gg + 1, :])
            gsr[gg] = scr

        pool_items = []
        for j, gg in enumerate(DVE_G):
            pool_items.append((10.2 + 4.9 * j - 1.5, ("b", gg, 0)))
        for gg in POOL_G:
            for half in range(2):
                t_need = 4.6 + 0.875 * gen_pos[2 * gg + half]
                if half == 0:
                    pool_items.append((t_need - 5.8, ("bp", gg, 0)))
                pool_items.append((t_need - 4.3, ("g", gg, half)))
        pool_items.sort(key=lambda s: s[0])
        gbt = {}
        for _, item in pool_items:
            kind, gg, half = item
            if kind in ("b", "bp"):
                t = gp.tile([128, BLOC], BF16, name=f"gb{gg}", tag="gb",
                            bufs=6)
                nc.gpsimd.partition_broadcast(t[:], gsr[gg][:])
                gbt[gg] = t
            else:
                gen_item(nc.gpsimd, gg, gbt[gg], half)
        for gg in DVE_G:
            gen_item(nc.vector, gg, gbt[gg], 0)
            gen_item(nc.vector, gg, gbt[gg], 1)

        def chunk(ap, c):
            return ap[:, c * 128:(c + 1) * 128]

        # ---- sigma pass ----
        psg = {}
        for c in range(NMC):
            psg[c] = pp.tile([128, CD], F32, name=f"psg{c}", tag="acc", bufs=8)
            nc.tensor.matmul(psg[c][:], gt8[:, :, c * 128:(c + 1) * 128],
                             bsg, start=True, stop=False, perf_mode=DR)
        for pos in range(NDK - QD):
            wt = wsg_t[pos // QD]
            for c in range(NMC):
                nc.tensor.matmul(
                    psg[c][:], zt[:, pos, :, c * 128:(c + 1) * 128],
                    wt[:, pos % QD], start=False, stop=False, perf_mode=DR)
        # Last positions chunk-major: per-chunk stops stagger, so drains
        # and the mu pass's bank takeovers begin while sigma finishes.
        for c in range(NMC):
            for pos in range(NDK - QD, NDK):
                nc.tensor.matmul(
                    psg[c][:], zt[:, pos, :, c * 128:(c + 1) * 128],
                    wsg_t[pos // QD][:, pos % QD], start=False,
                    stop=(pos == NDK - 1), perf_mode=DR)

        # sigma drain stage 1 — free the banks fast, in ring order, using
        # both ACT and DVE: even chunks Exp on ACT, odd chunks a bf16 copy
        # on DVE (its z-gen work is done by now). softplus finishes on ACT
        # during the mu pass.
        stg = {}
        osg = op.tile([128, NMC, CD], BF16, name="osg", tag="osg")
        prev_even = None
        for c in range(NMC):
            if c % 2 == 0:
                et = op.tile([128, CD], F32, name=f"et{c}", tag="et", bufs=2)
                nc.scalar.activation(et[:], psg[c][:],
                                     mybir.ActivationFunctionType.Exp,
                                     scale=1.0 / SC)
                if prev_even is not None:
                    pc, pet = prev_even
                    nc.scalar.activation(osg[:, pc], pet[:],
                                         mybir.ActivationFunctionType.Ln,
                                         bias=1.0)
                prev_even = (c, et)
            else:
                st = op.tile([128, CD], BF16, name=f"st{c}", tag="st", bufs=4)
                nc.vector.tensor_copy(st[:], psg[c][:])
                stg[c] = st

        # ---- mu pass; first dk-group chunk-major, chasing the bank frees ----
        pmu = {}
        wmu_t = {}
        wt0 = wp.tile([128, QD, 2, CD], FP8, name="wmu0", tag="w")
        nc.sync.dma_start(wt0[:], wmu_d[:, 0:QD])
        wmu_t[0] = wt0
        for c in range(NMC):
            pmu[c] = pp.tile([128, CD], F32, name=f"pmu{c}", tag="acc", bufs=8)
            nc.tensor.matmul(pmu[c][:], gt8[:, :, c * 128:(c + 1) * 128],
                             bmu, start=True, stop=False, perf_mode=DR)
            for pos in range(QD):
                nc.tensor.matmul(
                    pmu[c][:], zt[:, pos, :, c * 128:(c + 1) * 128],
                    wt0[:, pos], start=False, stop=False, perf_mode=DR)
        for q in range(1, NQ):
            wt = wp.tile([128, QD, 2, CD], FP8, name=f"wmu{q}", tag="w")
            nc.sync.dma_start(wt[:], wmu_d[:, q * QD:(q + 1) * QD])
            wmu_t[q] = wt
            for r in range(QD):
                pos = q * QD + r
                if pos < NDK - QD:
                    for c in range(NMC):
                        nc.tensor.matmul(
                            pmu[c][:], zt[:, pos, :, c * 128:(c + 1) * 128],
                            wt[:, r], start=False, stop=False, perf_mode=DR)
        for c in range(NMC):
            for pos in range(NDK - QD, NDK):
                nc.tensor.matmul(
                    pmu[c][:], zt[:, pos, :, c * 128:(c + 1) * 128],
                    wmu_t[pos // QD][:, pos % QD], start=False,
                    stop=(pos == NDK - 1), perf_mode=DR)

        # sigma drain stage 2 + store (ACT and DMA are free during mu).
        pc, pet = prev_even
        nc.scalar.activation(osg[:, pc], pet[:],
                             mybir.ActivationFunctionType.Ln, bias=1.0)
        for c in range(1, NMC, 2):
            et = op.tile([128, CD], F32, name=f"eo{c}", tag="et", bufs=2)
            nc.scalar.activation(et[:], stg[c][:],
                                 mybir.ActivationFunctionType.Exp,
                                 scale=1.0 / SC)
            nc.scalar.activation(osg[:, c], et[:],
                                 mybir.ActivationFunctionType.Ln, bias=1.0)
        out_sg = out_d[:, C + CD:].rearrange("(c p) o -> p c o", c=NMC)
        nc.gpsimd.dma_start(out_sg, osg[:])

        # mu drain: chunks 0-3 on ACT, 4-7 on DVE; stores in two halves on
        # the by-now-idle sync queue.
        omu = op.tile([128, NMC, CD], BF16, name="omu", tag="omu")
        out_mu = out_d[:, C:C + CD].rearrange("(c p) o -> p c o", c=NMC)
        for c in range(4):
            nc.scalar.activation(omu[:, c], pmu[c][:],
                                 mybir.ActivationFunctionType.Copy,
                                 scale=1.0 / SC)
            nc.vector.tensor_scalar_mul(omu[:, c + 4], pmu[c + 4][:],
                                        1.0 / SC)
            nc.sync.dma_start(out_mu[:, c:c + 1], omu[:, c:c + 1])
            nc.sync.dma_start(out_mu[:, c + 4:c + 5], omu[:, c + 4:c + 5])

        # ---- pi pass: all 8 chunks packed into one recycled PSUM bank ----
        ppi = pp.tile([128, CD], F32, name="ppi", tag="acc", bufs=8)
        for c in range(NMC):
            # start=True on c==0 marks the whole bank pending-zero; later
            # chunks' first writes land on pending-zero bytes.
            nc.tensor.matmul(ppi[:, c * C:(c + 1) * C],
                             gt8[:, :, c * 128:(c + 1) * 128], bpi,
                             start=(c == 0), stop=False, perf_mode=DR,
                             skip_group_check=True)
        for pos in range(NDK):
            for c in range(NMC):
                nc.tensor.matmul(
                    ppi[:, c * C:(c + 1) * C],
                    zt[:, pos, :, c * 128:(c + 1) * 128],
                    wpi[:, pos], start=False, stop=(pos == NDK - 1),
                    perf_mode=DR, skip_group_check=True)

        opi = op.tile([128, NMC * C], BF16, name="opi", tag="opi")
        nc.scalar.activation(opi[:], ppi[:, :NMC * C],
                             mybir.ActivationFunctionType.Copy, scale=1.0 / SC)
        out_pi = out_d[:, 0:C].rearrange("(c p) o -> p c o", c=NMC)
        nc.sync.dma_start(out_pi, opi[:].rearrange("p (c o) -> p c o", c=NMC))

    nc.compile()
    _cache["nc"] = nc
    return nc


def _prep_shared(W_mu, b_mu, W_sigma, b_sigma, W_pi, b_pi):
    fp8 = ml_dtypes.float8_e4m3
    bf16 = ml_dtypes.bfloat16
    w_cat = np.concatenate([W_pi, W_mu, W_sigma], axis=-1)      # [G, I, 1040]
    # k = g*512 + i -> (dt, j, p); permute dt into processing order and
    # store partition-major [p, pos, j, o].
    w8 = (w_cat.reshape(NDK, 2, 128, OUT_W) * WS)[PERM]
    w8 = np.ascontiguousarray(w8.transpose(2, 0, 1, 3)).astype(fp8)
    wpi = np.ascontiguousarray(w8[:, :, :, 0:C])
    wmu = np.ascontiguousarray(w8[:, :, :, C:C + CD])
    wsg = np.ascontiguousarray(w8[:, :, :, C + CD:])
    # Bias rows in fp8 DoubleRow layout [p, j, o] with group gg = 16j + p,
    # matching the gate operand of the PSUM bias matmuls.
    bcat = np.concatenate([b_pi, b_mu, b_sigma], axis=-1) * WS  # [G, 1040]
    b_dr = bcat.reshape(2, 16, OUT_W).transpose(1, 0, 2)        # [16, 2, 1040]
    return wpi, wmu, wsg, b_dr


_SDT = [dt for pos, dt in enumerate(PERM) if ROLES[pos] == "S"]


def _core_inputs(x, g, shared, c):
    fp8 = ml_dtypes.float8_e4m3
    bf16 = ml_dtypes.bfloat16
    wpi, wmu, wsg, b_dr = shared
    xs = x[c * BLOC:(c + 1) * BLOC]
    gs = g[c * BLOC:(c + 1) * BLOC]
    # z[b, k=(g,i)] = g[b,g]*x[b,i] for the streamed dk-tiles only, in
    # processing order, stored [p, spos, j, b].
    z3 = (gs[:, :, None] * xs[:, None, :]).reshape(BLOC, NDK, 2, 128)
    z = np.ascontiguousarray(
        (z3[:, _SDT] * ZS).transpose(3, 1, 2, 0)).astype(fp8)
    # x^T in [p, ib, b] layout for on-chip z-gen, pre-scaled by ZS and
    # shipped fp8 (halves its slice of the DMA-starved pass-1 prologue).
    xT = np.ascontiguousarray(
        (xs.T.reshape(4, 128, BLOC) * ZS).transpose(1, 0, 2)).astype(fp8)
    gT = np.ascontiguousarray(gs.T.astype(bf16))                # [32, 1024]
    # fp8 DoubleRow gate rows (gg = 16j + p, scaled like z) packed with
    # the bias rows: one startup DMA feeds every PSUM bias matmul.
    g_dr = (gs.T.reshape(2, 16, BLOC) * ZS).transpose(1, 0, 2)  # [16, 2, 1024]
    gb8 = np.ascontiguousarray(
        np.concatenate([g_dr, b_dr], axis=-1)).astype(fp8)      # [16,2,2064]
    return {"z": z, "wpi": wpi, "wmu": wmu, "wsg": wsg, "xt": xT,
            "gtb": gT, "gb8": gb8}


def kernel(x, g, W_mu, b_mu, W_sigma, b_sigma, W_pi, b_pi):
    nc = _build_program()
    shared = _prep_shared(W_mu, b_mu, W_sigma, b_sigma, W_pi, b_pi)
    in_maps = [_core_inputs(x, g, shared, c) for c in range(NCORES)]
    res = run_bass_kernel_spmd(nc, in_maps, core_ids=list(range(NCORES)))
    out = np.concatenate(
        [res.results[c]["out"].astype(np.float32) for c in range(NCORES)],
        axis=0)
    return np.ascontiguousarray(out)


# revision 42
# speedup vs baseline: 1.0041x; 1.0041x over previous
"""GroupGMM Trainium2 kernel (fp8 DoubleRow version).

Computes, for B=8192 samples with soft group-mixture weights over G=32 groups:
    logits = einsum("bi,gio,bg->bo", x, W_pi, g) + g @ b_pi        [B, 16]
    loc    = einsum(... W_mu ...)   + g @ b_mu                     [B, 512]
    scale  = softplus(einsum(... W_sigma ...) + g @ b_sigma)+1e-7  [B, 512]
    out    = concat([logits, loc, scale], -1)                      [B, 1040]

Strategy: data-parallel over batch across 8 NeuronCores (1024 rows each).
The group einsum folds into one matmul with contraction K = G*I = 16384 via
z[(g,i),b] = g[b,g]*x[b,i]. z and the weights are fp8e4 (scaled by 8 resp.
16 to dodge fp8 subnormals; the 1/128 is folded into the drains), so the PE
runs DoubleRow fp8 matmuls: 256-deep contraction per instruction at 0.5
cycles/row — 4x the bf16 matmul rate.

Three column passes over the 64 double-K tiles, with z resident in SBUF
after pass 1: sigma (8 PSUM banks, one per 128-sample chunk), then mu
(banks recycle as sigma drains), then pi (8 chunks packed in one recycled
bank). Pass 1 would be DMA-bound (z 16.8MB + W_sigma 8.4MB vs ~56us of
matmul), so z for 13 of the 32 groups is built on-chip instead of
streamed: the otherwise-idle Pool engine replicates gate rows across
partitions (partition_broadcast) and DVE (10 groups) plus Pool itself
(3 groups) multiply them into x^T tiles. PSUM accumulation commutes, so
the dk-tiles are processed in a host-chosen PERMUTED order (the host lays
W and z out in processing order): a greedy schedule interleaves streamed
and generated tiles so the DMA stream, DVE, and Pool all stay just ahead
of the PE. The g@b bias terms are accumulated directly in PSUM by a small
bf16 matmul (K=32) that opens each bank's accumulation group. The last
dk-positions of each pass run chunk-major so per-chunk stops (drains,
bank takeovers) stagger instead of bursting. softplus = Ln(Exp(v/128)+1)
on ACT; mu drains split ACT/DVE into bf16; outputs leave in four batched
bf16 stores and the host casts to f32.
"""

import numpy as np
import ml_dtypes

import concourse.bass as bass
import concourse.tile as tile
from concourse import bacc, mybir
from concourse.bass_utils import run_bass_kernel_spmd

B, I, G, C, D = 8192, 512, 32, 16, 32
CD = C * D                      # 512
OUT_W = C + 2 * CD              # 1040
NCORES = 8
BLOC = B // NCORES              # 1024
KTOT = G * I                    # 16384
NDK = KTOT // 256               # 64 double-K tiles (256-deep each)
NMC = BLOC // 128               # 8 sample chunks per core
ZS, WS = 8.0, 16.0              # fp8 pre-scales; drains divide by ZS*WS
SC = ZS * WS

# On-chip-generated groups (renumbering is free: the host permutes layouts).
N_DVE_G = 10
N_POOL_G = 3
STREAM_G = list(range(G - N_DVE_G - N_POOL_G))        # groups 0..18
DVE_G = list(range(len(STREAM_G), len(STREAM_G) + N_DVE_G))    # 19..28
POOL_G = list(range(DVE_G[-1] + 1, G))                         # 29..31
N_S = 2 * len(STREAM_G)         # 38 streamed dk-tiles

BF16 = mybir.dt.bfloat16
F32 = mybir.dt.float32
FP8 = mybir.dt.float8e4
DR = mybir.MatmulPerfMode.DoubleRow

_cache: dict = {}


def _schedule():
    """Greedy processing order for pass 1. Returns (perm, roles) where
    perm[pos] = dk-tile index (dt) processed at position pos and
    roles[pos] in {'S','D','P'}. Streamed dts are 0..N_S-1 in order; DVE
    dts are 2*DVE_G[0].. in order; Pool dts likewise. Cost constants are
    scheduling heuristics only — correctness never depends on them."""
    PE0, PEDT = 4.6, 0.875      # PE start and per-position matmul time
    WDMA, ZDMA = 0.364, 0.728   # us per position of W / of streamed z
    DVE0, DVEG = 8.7, 2.3       # DVE first-gen start, per-gen time
    # Pool per-gen cost is padded for the ~1.4us broadcasts it interleaves.
    POOL0, POOLG = 8.2, 6.3
    ns, nd, np_ = 0, 0, 0
    dma_t, pe_t = 1.0, PE0
    dve_t, pool_t = DVE0, POOL0
    perm, roles = [], []
    for pos in range(NDK):
        # Generated tiles are rate-limited: consume them the moment they
        # are ready (it costs nothing and spares stream DMA for later);
        # fall back to the stream, then to whatever finishes first.
        cand = []
        if nd < 2 * N_DVE_G:
            cand.append((max(dve_t + DVEG, dma_t + WDMA, pe_t), 0, "D"))
        if np_ < 2 * N_POOL_G:
            cand.append((max(pool_t + POOLG, dma_t + WDMA, pe_t), 1, "P"))
        if ns < N_S:
            cand.append((max(dma_t + ZDMA + WDMA, pe_t), 2, "S"))
        free = [c for c in cand if c[0] <= pe_t + PEDT + 1e-9]
        ready, _, role = min(free) if free else min(cand)
        roles.append(role)
        if role == "S":
            perm.append(ns)
            ns += 1
            dma_t += ZDMA + WDMA
        elif role == "D":
            perm.append(2 * DVE_G[0] + nd)
            nd += 1
            dve_t = max(dve_t, pe_t - DVEG) + DVEG
            dma_t += WDMA
        else:
            perm.append(2 * POOL_G[0] + np_)
            np_ += 1
            pool_t = max(pool_t, pe_t - POOLG) + POOLG
            dma_t += WDMA
        pe_t = max(pe_t + PEDT, ready)
    return perm, roles


PERM, ROLES = _schedule()
POS_OF_DT = {dt: pos for pos, dt in enumerate(PERM)}


def _z_runs():
    """Contiguous streamed-position runs, capped at 4, first two capped at
    2 for a fast pipeline start. Streamed positions map to consecutive
    z_d indices, so every run is one contiguous DMA on both sides."""
    runs = []
    pos = 0
    while pos < NDK:
        if ROLES[pos] != "S":
            pos += 1
            continue
        end = pos
        cap = 1 if len(runs) < 2 else (2 if len(runs) < 4 else 4)
        while (end + 1 < NDK and ROLES[end + 1] == "S" and end + 1 - pos < cap):
            end += 1
        runs.append((pos, end + 1))
        pos = end + 1
    return runs


def _build_program():
    if "nc" in _cache:
        return _cache["nc"]
    from contextlib import ExitStack

    nc = bacc.Bacc("TRN2", target_bir_lowering=False, debug=False)

    # Host tensors are "partition-major" [128, ...] and already permuted
    # into processing order; z_d holds only the streamed positions.
    z_d = nc.dram_tensor("z", [128, N_S, 2, BLOC], FP8, kind="ExternalInput")
    wmu_d = nc.dram_tensor("wmu", [128, NDK, 2, CD], FP8, kind="ExternalInput")
    wsg_d = nc.dram_tensor("wsg", [128, NDK, 2, CD], FP8, kind="ExternalInput")
    wpi_d = nc.dram_tensor("wpi", [128, NDK, 2, C], FP8, kind="ExternalInput")
    xt_d = nc.dram_tensor("xt", [128, 4, BLOC], FP8, kind="ExternalInput")
    # gate rows (bf16, for staging/broadcast) packed with the fp8
    # DoubleRow-layout gate/bias operands for the PSUM bias matmuls.
    gtb_d = nc.dram_tensor("gtb", [G, BLOC], BF16, kind="ExternalInput")
    gb8_d = nc.dram_tensor("gb8", [16, 2, BLOC + OUT_W], FP8,
                           kind="ExternalInput")
    out_d = nc.dram_tensor("out", [BLOC, OUT_W], BF16, kind="ExternalOutput")

    # Position of each engine's i-th generated tile, and first-gen position
    # per on-chip group (for broadcast pacing).
    dve_pos = [p for p in range(NDK) if ROLES[p] == "D"]
    pool_pos = [p for p in range(NDK) if ROLES[p] == "P"]
    gen_pos = {}
    for i, p in enumerate(dve_pos):
        gen_pos[2 * DVE_G[0] + i] = p
    for i, p in enumerate(pool_pos):
        gen_pos[2 * POOL_G[0] + i] = p

    with tile.TileContext(nc) as tc, ExitStack() as ctx:
        res = ctx.enter_context(tc.tile_pool(name="res", bufs=1))
        gp = ctx.enter_context(tc.tile_pool(name="gp", bufs=6))
        wp = ctx.enter_context(tc.tile_pool(name="wp", bufs=3))
        op = ctx.enter_context(tc.tile_pool(name="op", bufs=1))
        pp = ctx.enter_context(tc.tile_pool(name="pp", bufs=1, space="PSUM"))

        gb8 = res.tile([16, 2, BLOC + OUT_W], FP8, name="gb8", tag="gb8")
        nc.sync.dma_start(gb8[:], gb8_d[:])
        gt8 = gb8[:, :, 0:BLOC]
        bpi = gb8[:, :, BLOC:BLOC + C]
        bmu = gb8[:, :, BLOC + C:BLOC + C + CD]
        bsg = gb8[:, :, BLOC + C + CD:]

        zt = res.tile([128, NDK, 2, BLOC], FP8, name="zt", tag="zt")
        xtr = res.tile([128, 4, BLOC], FP8, name="xtr", tag="xtr")

        # Sigma-pass stream on the in-order sync queue, keyed by the
        # position that first needs each item.
        QD = 4
        NQ = NDK // QD
        onchip = DVE_G + POOL_G
        first_need = {gg: min(gen_pos[2 * gg], gen_pos[2 * gg + 1])
                      for gg in onchip}
        stream = []
        sidx = {}
        for i, (a, bnd) in enumerate(_z_runs()):
            stream.append((a, 0, ("z", a, bnd)))
        for q in range(1, NQ):
            stream.append((q * QD + 0.5, 0, ("w", q)))
        stream.append((0.4, 0, ("wh", 0)))
        stream.append((1.6, 0, ("wh", 1)))
        # x^T feeds every on-chip multiply: get it in right behind the
        # first z/w items, before the stream saturates.
        stream.append((1.5, 0, ("xt", 0)))
        stream.append((3.0, 0, ("xt", 1)))
        # Gate-row staging (2KB each): ahead of the group's broadcast,
        # paced so a 4-deep ring never blocks the queue. The two earliest
        # groups are staged via SWDGE on the idle Pool queue instead (no
        # HWDGE slot, no delay to the z/w stream).
        early_gs = sorted(onchip, key=lambda gg: first_need[gg])[:2]
        for gg in onchip:
            if gg not in early_gs:
                stream.append((max(1.2, first_need[gg] - 7), 1, ("gs", gg)))
        stream.sort(key=lambda s: (s[0], s[1]))

        wsg_t = {}
        gsr = {}
        for _, _, item in stream:
            if item[0] == "z":
                _, a, bnd = item
                s0 = sum(1 for p in range(a) if ROLES[p] == "S")
                nc.sync.dma_start(zt[:, a:bnd], z_d[:, s0:s0 + (bnd - a)])
            elif item[0] == "w":
                q = item[1]
                wt = wp.tile([128, QD, 2, CD], FP8, name=f"wsg{q}", tag="w")
                nc.sync.dma_start(wt[:], wsg_d[:, q * QD:(q + 1) * QD])
                wsg_t[q] = wt
            elif item[0] == "wh":
                h = item[1]
                if h == 0:
                    wsg_t[0] = wp.tile([128, QD, 2, CD], FP8, name="wsg0",
                                       tag="w")
                nc.sync.dma_start(wsg_t[0][:, 2 * h:2 * h + 2],
                                  wsg_d[:, 2 * h:2 * h + 2])
            elif item[0] == "gs":
                gg = item[1]
                scr = gp.tile([1, BLOC], BF16, name=f"gsr{gg}", tag="gsr",
                              bufs=5)
                nc.sync.dma_start(scr[:], gtb_d[gg:gg + 1, :])
                gsr[gg] = scr
            else:
                h = item[1]
                nc.sync.dma_start(xtr[:, 2 * h:2 * h + 2],
                                  xt_d[:, 2 * h:2 * h + 2])
        # Loads first needed later, queued behind the sigma stream.
        wpi = res.tile([128, NDK, 2, C], FP8, name="wpi", tag="wpi")
        nc.sync.dma_start(wpi[:], wpi_d[:])

        # On-chip z generation, emitted on Pool in need order: DVE-group
        # broadcasts must lead DVE's serial gen queue; Pool's own pairs
        # (broadcast + two multiplies) slot in between by position.
        def gen_item(eng, gg, t, half):
            dt = 2 * gg + half
            ib = 2 * half
            gb = t[:].unsqueeze(1).broadcast_to([128, 2, BLOC])
            eng.tensor_mul(zt[:, gen_pos[dt]], xtr[:, ib:ib + 2], gb)

        for gg in early_gs:
            scr = gp.tile([1, BLOC], BF16, name=f"gsre{gg}", tag="gsre",
                          bufs=2)
            nc.gpsimd.dma_start(scr[:], gtb_d[gg:gg + 1, :])
            gsr[gg] = scr

        pool_items = []
        for j, gg in enumerate(DVE_G):
            pool_items.append((10.2 + 4.9 * j - 1.5, ("b", gg, 0)))
        for gg in POOL_G:
            for half in range(2):
                t_need = 4.6 + 0.875 * gen_pos[2 * gg + half]
                if half == 0:
                    pool_items.append((t_need - 5.8, ("bp", gg, 0)))
                pool_items.append((t_need - 4.3, ("g", gg, half)))
        pool_items.sort(key=lambda s: s[0])
        gbt = {}
        for _, item in pool_items:
            kind, gg, half = item
            if kind in ("b", "bp"):
                t = gp.tile([128, BLOC], BF16, name=f"gb{gg}", tag="gb",
                            bufs=6)
                nc.gpsimd.partition_broadcast(t[:], gsr[gg][:])
                gbt[gg] = t
            else:
                gen_item(nc.gpsimd, gg, gbt[gg], half)
        for gg in DVE_G:
            gen_item(nc.vector, gg, gbt[gg], 0)
            gen_item(nc.vector, gg, gbt[gg], 1)

        def chunk(ap, c):
            return ap[:, c * 128:(c + 1) * 128]

        # ---- sigma pass ----
        psg = {}
        for c in range(NMC):
            psg[c] = pp.tile([128, CD], F32, name=f"psg{c}", tag="acc", bufs=8)
            nc.tensor.matmul(psg[c][:], gt8[:, :, c * 128:(c + 1) * 128],
                             bsg, start=True, stop=False, perf_mode=DR)
        for pos in range(NDK - QD):
            wt = wsg_t[pos // QD]
            for c in range(NMC):
                nc.tensor.matmul(
                    psg[c][:], zt[:, pos, :, c * 128:(c + 1) * 128],
                    wt[:, pos % QD], start=False, stop=False, perf_mode=DR)
        # Last positions chunk-major: per-chunk stops stagger, so drains
        # and the mu pass's bank takeovers begin while sigma finishes.
        for c in range(NMC):
            for pos in range(NDK - QD, NDK):
                nc.tensor.matmul(
                    psg[c][:], zt[:, pos, :, c * 128:(c + 1) * 128],
                    wsg_t[pos // QD][:, pos % QD], start=False,
                    stop=(pos == NDK - 1), perf_mode=DR)

        # sigma drain stage 1 — free the banks fast, in ring order, using
        # both ACT and DVE: even chunks Exp on ACT, odd chunks a bf16 copy
        # on DVE (its z-gen work is done by now). softplus finishes on ACT
        # during the mu pass.
        stg = {}
        osg = op.tile([128, NMC, CD], BF16, name="osg", tag="osg")
        prev_even = None
        for c in range(NMC):
            if c % 2 == 0:
                et = op.tile([128, CD], F32, name=f"et{c}", tag="et", bufs=2)
                nc.scalar.activation(et[:], psg[c][:],
                                     mybir.ActivationFunctionType.Exp,
                                     scale=1.0 / SC)
                if prev_even is not None:
                    pc, pet = prev_even
                    nc.scalar.activation(osg[:, pc], pet[:],
                                         mybir.ActivationFunctionType.Ln,
                                         bias=1.0)
                prev_even = (c, et)
            else:
                st = op.tile([128, CD], BF16, name=f"st{c}", tag="st", bufs=4)
                nc.vector.tensor_copy(st[:], psg[c][:])
                stg[c] = st

        # ---- mu pass; first dk-group chunk-major, chasing the bank frees ----
        pmu = {}
        wmu_t = {}
        wt0 = wp.tile([128, QD, 2, CD], FP8, name="wmu0", tag="w")
        nc.sync.dma_start(wt0[:], wmu_d[:, 0:QD])
        wmu_t[0] = wt0
        for c in range(NMC):
            pmu[c] = pp.tile([128, CD], F32, name=f"pmu{c}", tag="acc", bufs=8)
            nc.tensor.matmul(pmu[c][:], gt8[:, :, c * 128:(c + 1) * 128],
                             bmu, start=True, stop=False, perf_mode=DR)
            for pos in range(QD):
                nc.tensor.matmul(
                    pmu[c][:], zt[:, pos, :, c * 128:(c + 1) * 128],
                    wt0[:, pos], start=False, stop=False, perf_mode=DR)
        for q in range(1, NQ):
            wt = wp.tile([128, QD, 2, CD], FP8, name=f"wmu{q}", tag="w")
            nc.sync.dma_start(wt[:], wmu_d[:, q * QD:(q + 1) * QD])
            wmu_t[q] = wt
            for r in range(QD):
                pos = q * QD + r
                if pos < NDK - QD:
                    for c in range(NMC):
                        nc.tensor.matmul(
                            pmu[c][:], zt[:, pos, :, c * 128:(c + 1) * 128],
                            wt[:, r], start=False, stop=False, perf_mode=DR)
        for c in range(NMC):
            for pos in range(NDK - QD, NDK):
                nc.tensor.matmul(
                    pmu[c][:], zt[:, pos, :, c * 128:(c + 1) * 128],
                    wmu_t[pos // QD][:, pos % QD], start=False,
                    stop=(pos == NDK - 1), perf_mode=DR)

        # sigma drain stage 2 + store (ACT and DMA are free during mu).
        pc, pet = prev_even
        nc.scalar.activation(osg[:, pc], pet[:],
                             mybir.ActivationFunctionType.Ln, bias=1.0)
        for c in range(1, NMC, 2):
            et = op.tile([128, CD], F32, name=f"eo{c}", tag="et", bufs=2)
            nc.scalar.activation(et[:], stg[c][:],
                                 mybir.ActivationFunctionType.Exp,
                                 scale=1.0 / SC)
            nc.scalar.activation(osg[:, c], et[:],
                                 mybir.ActivationFunctionType.Ln, bias=1.0)
        out_sg = out_d[:, C + CD:].rearrange("(c p) o -> p c o", c=NMC)
        nc.gpsimd.dma_start(out_sg, osg[:])

        # mu drain: chunks 0-3 on ACT, 4-7 on DVE; stores in two halves on
        # the by-now-idle sync queue.
        omu = op.tile([128, NMC, CD], BF16, name="omu", tag="omu")
        out_mu = out_d[:, C:C + CD].rearrange("(c p) o -> p c o", c=NMC)
        for c in range(4):
            nc.scalar.activation(omu[:, c], pmu[c][:],
                                 mybir.ActivationFunctionType.Copy,
                                 scale=1.0 / SC)
            nc.vector.tensor_scalar_mul(omu[:, c + 4], pmu[c + 4][:],
                                        1.0 / SC)
            nc.sync.dma_start(out_mu[:, c:c + 1], omu[:, c:c + 1])
            nc.sync.dma_start(out_mu[:, c + 4:c + 5], omu[:, c + 4:c + 5])

        # ---- pi pass: all 8 chunks packed into one recycled PSUM bank ----
        ppi = pp.tile([128, CD], F32, name="ppi", tag="acc", bufs=8)
        for c in range(NMC):
            # start=True on c==0 marks the whole bank pending-zero; later
            # chunks' first writes land on pending-zero bytes.
            nc.tensor.matmul(ppi[:, c * C:(c + 1) * C],
                             gt8[:, :, c * 128:(c + 1) * 128], bpi,
                             start=(c == 0), stop=False, perf_mode=DR,
                             skip_group_check=True)
        for pos in range(NDK):
            for c in range(NMC):
                nc.tensor.matmul(
                    ppi[:, c * C:(c + 1) * C],
                    zt[:, pos, :, c * 128:(c + 1) * 128],
                    wpi[:, pos], start=False, stop=(pos == NDK - 1),
                    perf_mode=DR, skip_group_check=True)

        opi = op.tile([128, NMC * C], BF16, name="opi", tag="opi")
        nc.scalar.activation(opi[:], ppi[:, :NMC * C],
                             mybir.ActivationFunctionType.Copy, scale=1.0 / SC)
        out_pi = out_d[:, 0:C].rearrange("(c p) o -> p c o", c=NMC)
        nc.sync.dma_start(out_pi, opi[:].rearrange("p (c o) -> p c o", c=NMC))

    nc.compile()
    _cache["nc"] = nc
    return nc


def _prep_shared(W_mu, b_mu, W_sigma, b_sigma, W_pi, b_pi):
    fp8 = ml_dtypes.float8_e4m3
    bf16 = ml_dtypes.bfloat16
    w_cat = np.concatenate([W_pi, W_mu, W_sigma], axis=-1)      # [G, I, 1040]
    # k = g*512 + i -> (dt, j, p); permute dt into processing order and
    # store partition-major [p, pos, j, o].
    w8 = (w_cat.reshape(NDK, 2, 128, OUT_W) * WS)[PERM]
    w8 = np.ascontiguousarray(w8.transpose(2, 0, 1, 3)).astype(fp8)
    wpi = np.ascontiguousarray(w8[:, :, :, 0:C])
    wmu = np.ascontiguousarray(w8[:, :, :, C:C + CD])
    wsg = np.ascontiguousarray(w8[:, :, :, C + CD:])
    # Bias rows in fp8 DoubleRow layout [p, j, o] with group gg = 16j + p,
    # matching the gate operand of the PSUM bias matmuls.
    bcat = np.concatenate([b_pi, b_mu, b_sigma], axis=-1) * WS  # [G, 1040]
    b_dr = bcat.reshape(2, 16, OUT_W).transpose(1, 0, 2)        # [16, 2, 1040]
    return wpi, wmu, wsg, b_dr


_SDT = [dt for pos, dt in enumerate(PERM) if ROLES[pos] == "S"]


def _core_inputs(x, g, shared, c):
    fp8 = ml_dtypes.float8_e4m3
    bf16 = ml_dtypes.bfloat16
    wpi, wmu, wsg, b_dr = shared
    xs = x[c * BLOC:(c + 1) * BLOC]
    gs = g[c * BLOC:(c + 1) * BLOC]
    # z[b, k=(g,i)] = g[b,g]*x[b,i] for the streamed dk-tiles only, in
    # processing order, stored [p, spos, j, b].
    z3 = (gs[:, :, None] * xs[:, None, :]).reshape(BLOC, NDK, 2, 128)
    z = np.ascontiguousarray(
        (z3[:, _SDT] * ZS).transpose(3, 1, 2, 0)).astype(fp8)
    # x^T in [p, ib, b] layout for on-chip z-gen, pre-scaled by ZS and
    # shipped fp8 (halves its slice of the DMA-starved pass-1 prologue).
    xT = np.ascontiguousarray(
        (xs.T.reshape(4, 128, BLOC) * ZS).transpose(1, 0, 2)).astype(fp8)
    gT = np.ascontiguousarray(gs.T.astype(bf16))                # [32, 1024]
    # fp8 DoubleRow gate rows (gg = 16j + p, scaled like z) packed with
    # the bias rows: one startup DMA feeds every PSUM bias matmul.
    g_dr = (gs.T.reshape(2, 16, BLOC) * ZS).transpose(1, 0, 2)  # [16, 2, 1024]
    gb8 = np.ascontiguousarray(
        np.concatenate([g_dr, b_dr], axis=-1)).astype(fp8)      # [16,2,2064]
    return {"z": z, "wpi": wpi, "wmu": wmu, "wsg": wsg, "xt": xT,
            "gtb": gT, "gb8": gb8}


def kernel(x, g, W_mu, b_mu, W_sigma, b_sigma, W_pi, b_pi):
    nc = _build_program()
    shared = _prep_shared(W_mu, b_mu, W_sigma, b_sigma, W_pi, b_pi)
    in_maps = [_core_inputs(x, g, shared, c) for c in range(NCORES)]
    res = run_bass_kernel_spmd(nc, in_maps, core_ids=list(range(NCORES)))
    out = np.concatenate(
        [res.results[c]["out"].astype(np.float32) for c in range(NCORES)],
        axis=0)
    return np.ascontiguousarray(out)
